# revision 10
# baseline (speedup 1.0000x reference)
"""Trainium2 Bass kernel for DyadicCrossAttention.

Sharding: 8 cores = 2 batches x 4 head-groups (2 heads of d=32 each, i.e. a
64-channel slice of HID=256). Each core computes its batch's q/k/v projections
for its 64 channels, both cross-attention directions for its 2 heads, and a
partial output projection Wo[:, ch] @ out[ch]. The host sums the 4 per-batch
partials and adds the constant bias vectors (bo + Wo @ bv, both independent of
the pixel, since softmax weights sum to 1).

Matmul operands are fp16 (fp32 PSUM accumulation). fp16 was chosen over bf16
for its 3 extra mantissa bits (rel err ~6e-4 end-to-end vs ~5e-3) at equal
matmul throughput; all tensors here are comfortably inside fp16 range.

Score layout is transposed (keys on partitions): St[n2, n1] = k^T q per head,
exp on the scalar engine with the 1/sqrt(d) scale fused, and the softmax
denominator obtained for free by appending a ones-column to v^T in the
attention*V matmul.
"""

import sys

import numpy as np

sys.path.insert(0, "/opt/trn_rl_repo")

B, DIM1, DIM2, HID, HEADS, H, W, OUT = 2, 256, 512, 256, 8, 48, 48, 256
HD = HID // HEADS  # 32
SCALE = float(HD) ** -0.5
N = H * W  # 2304
NJ = N // 128  # 18 key chunks
NTILES = [(0, 512), (512, 512), (1024, 512), (1536, 512), (2048, 256)]

MM_MODE = "f16"        # "f16", "bf16" or "f32r" matmul operand format
TRACE = False          # set by test.py for profiled runs
TRACE_KWARGS = {}
LAST_RESULTS = None    # BassKernelResults of the last run (for test.py)

_CACHE = {}


def _split_multiwait(nc, mybir, limit=1):
    """Walrus rejects instructions carrying >limit semaphore waits; move the
    excess onto InstNoOp instructions inserted just before on the same engine."""
    for f in nc.m.functions:
        for bb in f.blocks:
            out = []
            changed = False
            for inst in bb.instructions:
                si = inst.sync_info
                if si is not None and len(si.on_wait) > limit:
                    waits = list(si.on_wait)
                    pre, keep = waits[:-limit], waits[-limit:]
                    for ci in range(0, len(pre), limit):
                        nop = mybir.InstNoOp(
                            name=f"{inst.name}-ws{ci}", ins=[], outs=[]
                        )
                        nop.engine = inst.engine
                        nop.sync_info = mybir.SyncInfo(
                            on_wait=pre[ci : ci + limit], on_update=[]
                        )
                        out.append(nop)
                    inst.sync_info = mybir.SyncInfo(
                        on_wait=keep, on_update=list(si.on_update)
                    )
                    changed = True
                out.append(inst)
            if changed:
                bb.instructions = out


def _build(mm_mode=None, reps=1):
    import concourse.bass as bass
    import concourse.tile as tile
    from concourse import mybir

    if mm_mode is None:
        mm_mode = MM_MODE
    f32 = mybir.dt.float32
    f32r = mybir.dt.float32r
    md = {"bf16": mybir.dt.bfloat16, "f16": mybir.dt.float16, "f32r": f32r}[mm_mode]
    AF = mybir.ActivationFunctionType

    nc = bass.Bass()

    x1_d = nc.dram_tensor("x1", [DIM1, N], md, kind="ExternalInput")
    x2_d = nc.dram_tensor("x2", [DIM2, N], md, kind="ExternalInput")
    wqk1_d = nc.dram_tensor("wqk1", [DIM1, 128], md, kind="ExternalInput")
    wv1_d = nc.dram_tensor("wv1", [DIM1, 64], md, kind="ExternalInput")
    wqk2_d = nc.dram_tensor("wqk2", [DIM2, 128], md, kind="ExternalInput")
    wv2_d = nc.dram_tensor("wv2", [DIM2, 64], md, kind="ExternalInput")
    wo1a_d = nc.dram_tensor("wo1a", [32, OUT], md, kind="ExternalInput")
    wo1b_d = nc.dram_tensor("wo1b", [32, OUT], md, kind="ExternalInput")
    wo2a_d = nc.dram_tensor("wo2a", [32, OUT], md, kind="ExternalInput")
    wo2b_d = nc.dram_tensor("wo2b", [32, OUT], md, kind="ExternalInput")
    bqk1_d = nc.dram_tensor("bqk1", [128, 1], f32, kind="ExternalInput")
    bqk2_d = nc.dram_tensor("bqk2", [128, 1], f32, kind="ExternalInput")
    e33_d = nc.dram_tensor("e33", [33, 64], md, kind="ExternalInput")
    y1_d = nc.dram_tensor("y1", [OUT, N], f32, kind="ExternalOutput")
    y2_d = nc.dram_tensor("y2", [OUT, N], f32, kind="ExternalOutput")

    with tile.TileContext(nc) as tc, nc.allow_low_precision(
        reason="bf16 matmul operands by design; fp32 psum accumulation"
    ):
        with tc.tile_pool(name="const", bufs=1) as const:
            # ---- resident tensors ----
            # weights first (small, unblock the first projections), then the
            # activations in column-halves so projection ntile 0 can start
            # after ~1/2 of each chunk instead of the full 2304 columns.
            # DMA order = startup critical path: biases + qk weights first
            # (the first evictions wait on the bias tiles), then the first
            # column-halves of x so projection ntile 0 unlocks early.
            bqk1 = const.tile([128, 1], f32)
            nc.sync.dma_start(bqk1[:], bqk1_d[:])
            bqk2 = const.tile([128, 1], f32)
            nc.sync.dma_start(bqk2[:], bqk2_d[:])
            wqk1 = const.tile([128, 2, 128], md)
            for c in range(2):
                nc.sync.dma_start(wqk1[:, c, :], wqk1_d[c * 128 : (c + 1) * 128, :])
            wqk2 = const.tile([128, 4, 128], md)
            for c in range(4):
                nc.sync.dma_start(wqk2[:, c, :], wqk2_d[c * 128 : (c + 1) * 128, :])
            HN = N // 2
            x1s = const.tile([128, 2, N], md)
            x2s = const.tile([128, 4, N], md)
            for c in range(2):
                nc.sync.dma_start(x1s[:, c, 0:HN], x1_d[c * 128 : (c + 1) * 128, 0:HN])
            for c in range(4):
                nc.sync.dma_start(x2s[:, c, 0:HN], x2_d[c * 128 : (c + 1) * 128, 0:HN])
            for c in range(2):
                nc.sync.dma_start(x1s[:, c, HN:N], x1_d[c * 128 : (c + 1) * 128, HN:N])
            for c in range(4):
                nc.sync.dma_start(x2s[:, c, HN:N], x2_d[c * 128 : (c + 1) * 128, HN:N])
            wv1 = const.tile([128, 2, 64], md)
            for c in range(2):
                nc.sync.dma_start(wv1[:, c, :], wv1_d[c * 128 : (c + 1) * 128, :])
            wv2 = const.tile([128, 4, 64], md)
            for c in range(4):
                nc.sync.dma_start(wv2[:, c, :], wv2_d[c * 128 : (c + 1) * 128, :])
            wo1a = const.tile([32, OUT], md)
            nc.sync.dma_start(wo1a[:], wo1a_d[:])
            wo1b = const.tile([32, OUT], md)
            nc.sync.dma_start(wo1b[:], wo1b_d[:])
            wo2a = const.tile([32, OUT], md)
            nc.sync.dma_start(wo2a[:], wo2a_d[:])
            wo2b = const.tile([32, OUT], md)
            nc.sync.dma_start(wo2b[:], wo2b_d[:])
            e33 = const.tile([33, 64], md)
            nc.sync.dma_start(e33[:], e33_d[:])

            # projections (channel-major q/k; pixel-major v with ones columns)
            q1s = const.tile([64, N], md)
            k1s = const.tile([64, N], md)
            q2s = const.tile([64, N], md)
            k2s = const.tile([64, N], md)
            # per key-chunk j, 128 columns: [v_h0(32) | 1 | pad] [v_h1(32) | 1 | pad]
            v1T = const.tile([128, NJ, 128], md)
            v2T = const.tile([128, NJ, 128], md)
            for vt in (v1T, v2T):
                for ones_col in (vt[:, :, 32:33], vt[:, :, 96:97]):
                    if mm_mode == "f32r":
                        ones_col = ones_col.bitcast(f32)
                    nc.vector.memset(ones_col, 1.0)
            # reciprocal staging: rows 0 and 32 live, the rest stays zero
            rdens = []
            for _ri in range(2):
                rd = const.tile([64, 512], md, name=f"rden{_ri}")
                nc.vector.memset(rd[:], 0.0)
                rdens.append(rd)

            # ---- phase 1: projections ----
            with (
                tc.tile_pool(name="pj", bufs=4, space="PSUM") as pj,
                tc.tile_pool(name="pv", bufs=4, space="PSUM") as pv,
            ):
                for xs, wqk, bqk, nch, qdst, kdst in (
                    (x1s, wqk1, bqk1, 2, q1s, k1s),
                    (x2s, wqk2, bqk2, 4, q2s, k2s),
                ):
                    for n0, nt in NTILES:
                        ps = pj.tile([128, 512], f32, tag="ps")
                        for c in range(nch):
                            nc.tensor.matmul(
                                ps[:, :nt],
                                wqk[:, c, :],
                                xs[:, c, n0 : n0 + nt],
                                start=(c == 0),
                                stop=(c == nch - 1),
                            )
                        nc.scalar.activation(
                            qdst[:, n0 : n0 + nt], ps[0:64, :nt],
                            AF.Identity, bias=bqk[0:64, :], scale=1.0,
                        )
                        nc.scalar.activation(
                            kdst[:, n0 : n0 + nt], ps[64:128, :nt],
                            AF.Identity, bias=bqk[64:128, :], scale=1.0,
                        )
                for xs, wv, nch, vdst in ((x1s, wv1, 2, v1T), (x2s, wv2, 4, v2T)):
                    for j in range(NJ):
                        pvt = pv.tile([128, 64], f32, tag="pvt")
                        for c in range(nch):
                            nc.tensor.matmul(
                                pvt[:],
                                xs[:, c, j * 128 : (j + 1) * 128],
                                wv[:, c, :],
                                start=(c == 0),
                                stop=(c == nch - 1),
                            )
                        nc.vector.tensor_copy(vdst[:, j, 0:32], pvt[:, 0:32])
                        nc.vector.tensor_copy(vdst[:, j, 64:96], pvt[:, 32:64])

            # ---- phase 2: attention + output projection ----
            with (
                tc.tile_pool(name="stp", bufs=2, space="PSUM") as stp,
                tc.tile_pool(name="u0p", bufs=2, space="PSUM") as u0p,
                tc.tile_pool(name="rbp", bufs=1, space="PSUM") as rbp,
                tc.tile_pool(name="opp", bufs=1, space="PSUM") as opp,
                tc.tile_pool(name="ptp", bufs=6) as ptp,
                tc.tile_pool(name="sbs", bufs=2) as sbs,
            ):
                blk = 0
                for qs, ks, vT, woa, wob, ydst in (
                    (q1s, k2s, v2T, wo1a, wo1b, y1_d),
                    (q2s, k1s, v1T, wo2a, wo2b, y2_d),
                ) * reps:
                    for n0, nt in NTILES:
                        rden = rdens[blk % 2]
                        blk += 1
                        ut = u0p.tile([128, 512], f32, tag="u0")
                        u0t = ut[0:33, :]
                        u1t = ut[64:97, :]
                        for j in range(NJ):
                            st = stp.tile([128, 2, 512], f32, tag="st")
                            nc.tensor.matmul(
                                st[:, 0, :nt],
                                ks[0:32, j * 128 : (j + 1) * 128],
                                qs[0:32, n0 : n0 + nt],
                            )
                            nc.tensor.matmul(
                                st[:, 1, :nt],
                                ks[32:64, j * 128 : (j + 1) * 128],
                                qs[32:64, n0 : n0 + nt],
                            )
                            pt = ptp.tile([128, 2, 512], md, tag="pt")
                            nc.scalar.activation(
                                pt[:, :, :nt], st[:, :, :nt], AF.Exp, scale=SCALE
                            )
                            nc.tensor.matmul(
                                u0t[:, :nt],
                                vT[:, j, 0:33],
                                pt[:, 0, :nt],
                                start=(j == 0),
                                stop=(j == NJ - 1),
                            )
                            nc.tensor.matmul(
                                u1t[:, :nt],
                                vT[:, j, 64:97],
                                pt[:, 1, :nt],
                                start=(j == 0),
                                stop=(j == NJ - 1),
                            )
                        # normalization: O_h = U_h[0:32] * (1 / U_h[32])
                        nc.vector.reciprocal(rden[0:1, :nt], u0t[32:33, :nt])
                        nc.vector.reciprocal(rden[32:33, :nt], u1t[32:33, :nt])
                        rbt = rbp.tile([64, 512], f32, tag="rb")
                        nc.tensor.matmul(rbt[:, :nt], e33[:], rden[0:33, :nt])
                        rb0 = sbs.tile([32, 512], f32, tag="rb0")
                        nc.vector.tensor_copy(rb0[:, :nt], rbt[0:32, :nt])
                        rb1 = sbs.tile([32, 512], f32, tag="rb1")
                        nc.vector.tensor_copy(rb1[:, :nt], rbt[32:64, :nt])
                        o0 = sbs.tile([32, 512], md, tag="o0")
                        nc.vector.tensor_mul(o0[:, :nt], u0t[0:32, :nt], rb0[:, :nt])
                        o1 = sbs.tile([32, 512], md, tag="o1")
                        nc.vector.tensor_mul(o1[:, :nt], u1t[0:32, :nt], rb1[:, :nt])
                        for mt in range(2):
                            opt = opp.tile([128, 512], f32, tag="op")
                            nc.tensor.matmul(
                                opt[:, :nt],
                                woa[:, mt * 128 : (mt + 1) * 128],
                                o0[:, :nt],
                                start=True,
                                stop=False,
                            )
                            nc.tensor.matmul(
                                opt[:, :nt],
                                wob[:, mt * 128 : (mt + 1) * 128],
                                o1[:, :nt],
                                start=False,
                                stop=True,
                            )
                            yt = sbs.tile([128, 512], f32, tag="yt")
                            nc.vector.tensor_copy(yt[:, :nt], opt[:, :nt])
                            nc.sync.dma_start(
                                ydst[mt * 128 : (mt + 1) * 128, n0 : n0 + nt],
                                yt[:, :nt],
                            )

    _split_multiwait(nc, mybir, limit=1)
    return nc


def _get_nc():
    key = ("nc", MM_MODE)
    if key not in _CACHE:
        _CACHE[key] = _build()
    return _CACHE[key]


def kernel(
    modal1_feat, modal2_feat, Wq1, bq1, Wk1, bk1, Wv1, bv1,
    Wq2, bq2, Wk2, bk2, Wv2, bv2, Wo1, bo1, Wo2, bo2,
):
    global LAST_RESULTS
    from concourse.bass_utils import run_bass_kernel_spmd

    if MM_MODE == "bf16":
        import ml_dtypes
        md_np = ml_dtypes.bfloat16
    elif MM_MODE == "f16":
        md_np = np.float16
    else:
        md_np = np.float32

    f = np.float32
    modal1_feat = np.asarray(modal1_feat, f)
    modal2_feat = np.asarray(modal2_feat, f)
    Wq1, bq1 = np.asarray(Wq1, f), np.asarray(bq1, f)
    Wk1, bk1 = np.asarray(Wk1, f), np.asarray(bk1, f)
    Wv1, bv1 = np.asarray(Wv1, f), np.asarray(bv1, f)
    Wq2, bq2 = np.asarray(Wq2, f), np.asarray(bq2, f)
    Wk2, bk2 = np.asarray(Wk2, f), np.asarray(bk2, f)
    Wv2, bv2 = np.asarray(Wv2, f), np.asarray(bv2, f)
    Wo1, bo1 = np.asarray(Wo1, f), np.asarray(bo1, f)
    Wo2, bo2 = np.asarray(Wo2, f), np.asarray(bo2, f)

    e33 = np.zeros((33, 64), f)
    e33[0, 0:32] = 1.0
    e33[32, 32:64] = 1.0

    def cvt(a):
        return np.ascontiguousarray(np.asarray(a, md_np))

    in_maps = []
    for core in range(8):
        b, hg = core // 4, core % 4
        ch = slice(hg * 64, hg * 64 + 64)
        cha = slice(hg * 64, hg * 64 + 32)
        chb = slice(hg * 64 + 32, hg * 64 + 64)
        in_maps.append({
            "x1": cvt(modal1_feat[b].reshape(DIM1, N)),
            "x2": cvt(modal2_feat[b].reshape(DIM2, N)),
            "wqk1": cvt(np.concatenate([Wq1[ch].T, Wk1[ch].T], axis=1)),
            "wv1": cvt(Wv1[ch].T),
            "wqk2": cvt(np.concatenate([Wq2[ch].T, Wk2[ch].T], axis=1)),
            "wv2": cvt(Wv2[ch].T),
            "wo1a": cvt(Wo1[:, cha].T),
            "wo1b": cvt(Wo1[:, chb].T),
            "wo2a": cvt(Wo2[:, cha].T),
            "wo2b": cvt(Wo2[:, chb].T),
            "bqk1": np.ascontiguousarray(
                np.concatenate([bq1[ch], bk1[ch]])[:, None]),
            "bqk2": np.ascontiguousarray(
                np.concatenate([bq2[ch], bk2[ch]])[:, None]),
            "e33": cvt(e33),
        })

    nc = _get_nc()
    res = run_bass_kernel_spmd(
        nc, in_maps, core_ids=list(range(8)), trace=TRACE, **TRACE_KWARGS
    )
    LAST_RESULTS = res

    out1 = np.zeros((B, OUT, N), f)
    out2 = np.zeros((B, OUT, N), f)
    for core in range(8):
        b = core // 4
        out1[b] += res.results[core]["y1"]
        out2[b] += res.results[core]["y2"]
    # constant (per-pixel-independent) bias terms: bo + Wo @ bv
    out1 += (bo1 + Wo1 @ bv2)[None, :, None]
    out2 += (bo2 + Wo2 @ bv1)[None, :, None]
    return (
        out1.reshape(B, OUT, H, W),
        out2.reshape(B, OUT, H, W),
    )
